# revision 15
# baseline (speedup 1.0000x reference)
import os
import sys

import numpy as np

sys.path.insert(0, "/opt/trn_rl_repo")

import ml_dtypes  # noqa: E402
from contextlib import ExitStack  # noqa: E402

import concourse.bass as bass  # noqa: E402
import concourse.bacc as bacc  # noqa: E402
import concourse.tile as tile  # noqa: E402
from concourse import mybir  # noqa: E402
from concourse.bass_utils import run_bass_kernel_spmd  # noqa: E402
from concourse.kernels.tile_matmul import make_identity  # noqa: E402

AF = mybir.ActivationFunctionType
ALU = mybir.AluOpType
AX = mybir.AxisListType
F32 = mybir.dt.float32
BF16 = mybir.dt.bfloat16

N_CORES = 8
B_FULL = 2048
BPC = B_FULL // N_CORES      # 256 batch rows per core
NTOK = 8
DIM = 1024
HID = 4096
H = 16                       # heads
HD = DIM // H                # 64 head dim
GE = HID // H                # 256 v-dim per head
SCALE = HD ** -0.5
LN_EPS = 1e-5

LAST_RESULT = None


def build_program(Wl, Ww, bl, bw, nbt=BPC // 128, use_silu=True):
    """Emit the per-core SPMD program. Token order within a 128-row b-tile is
    n-major: GEMM output tiles are [128 b, ...] for a fixed token n, which is
    exactly the layout the attention middle needs (batch in partitions)."""
    bpc = nbt * 128
    nc = bass.Bass("TRN2", target_bir_lowering=False, debug=False)
    x_d = nc.declare_dram_parameter("x", [bpc, NTOK * DIM], F32, isOutput=False)
    wq_d = nc.declare_dram_parameter("wq", [DIM, DIM], BF16, isOutput=False)
    wk_d = nc.declare_dram_parameter("wk", [DIM, DIM], BF16, isOutput=False)
    wv_d = nc.declare_dram_parameter("wv", [DIM, HID], BF16, isOutput=False)
    wp_d = nc.declare_dram_parameter("wp", [HID, DIM], BF16, isOutput=False)
    out_d = nc.declare_dram_parameter("out", [bpc, NTOK * DIM], F32, isOutput=True)

    with tile.TileContext(nc) as tc:
        with ExitStack() as ctx:
            ep = ctx.enter_context
            const_p = ep(tc.tile_pool(name="const", bufs=1))
            xld_p = ep(tc.tile_pool(name="xld", bufs=3))
            xa_p = ep(tc.tile_pool(name="xa", bufs=1))     # X^T / A^T (shared slot)
            wst_p = ep(tc.tile_pool(name="wst", bufs=2))   # weight stream chunks
            q_p = ep(tc.tile_pool(name="q", bufs=1))
            k_p = ep(tc.tile_pool(name="k", bufs=1))
            v_p = ep(tc.tile_pool(name="v", bufs=1))
            sc_p = ep(tc.tile_pool(name="sc", bufs=2))     # 4KiB scratch
            sm_p = ep(tc.tile_pool(name="sm", bufs=1))
            o_p = ep(tc.tile_pool(name="o", bufs=1))
            a_p = ep(tc.tile_pool(name="a", bufs=1))
            outsb_p = ep(tc.tile_pool(name="outsb", bufs=2))
            stat_p = ep(tc.tile_pool(name="stat", bufs=2))
            psum_mm = ep(tc.tile_pool(name="psum_mm", bufs=4, space="PSUM"))
            psum_tr = ep(tc.tile_pool(name="psum_tr", bufs=2, space="PSUM"))

            ident_f32 = const_p.tile([128, 128], F32)
            make_identity(nc, ident_f32)
            ident_bf = const_p.tile([128, 128], BF16)
            make_identity(nc, ident_bf)
            eps_t = const_p.tile([128, 1], F32)
            nc.vector.memset(eps_t, LN_EPS)

            for bt in range(nbt):
                b0 = bt * 128

                # ---- X^T: transpose x rows into [dim, batch] blocks per token
                xt_big = xa_p.tile([128, NTOK * 8, 128], BF16, tag="xa")
                for n in range(NTOK):
                    for i in range(8):
                        xb = xld_p.tile([128, 128], F32, tag="xld")
                        nc.sync.dma_start(
                            out=xb,
                            in_=x_d[b0:b0 + 128, n * DIM + i * 128:n * DIM + (i + 1) * 128],
                        )
                        pt = psum_tr.tile([128, 128], F32, tag="ptr")
                        nc.tensor.transpose(pt, xb, ident_f32)
                        nc.scalar.copy(xt_big[:, n * 8 + i, :], pt)

                # ---- QKV GEMMs (weights streamed in 512-wide chunks)
                q_big = q_p.tile([128, NTOK, DIM], BF16, tag="q")
                k_big = k_p.tile([128, NTOK, DIM], BF16, tag="k")
                v_big = v_p.tile([128, NTOK, HID], BF16, tag="v")
                gemms = [
                    (wq_d, DIM // 512, q_big, SCALE),
                    (wk_d, DIM // 512, k_big, 1.0),
                    (wv_d, HID // 512, v_big, 1.0),
                ]
                for w_d, njc, dst_big, scale in gemms:
                    for jc in range(njc):
                        wt = wst_p.tile([128, 8, 512], BF16, tag="w8")
                        for i in range(8):
                            nc.sync.dma_start(
                                out=wt[:, i, :],
                                in_=w_d[i * 128:(i + 1) * 128, jc * 512:(jc + 1) * 512],
                            )
                        for n in range(NTOK):
                            ps = psum_mm.tile([128, 512], F32, tag="mm")
                            for i in range(8):
                                nc.tensor.matmul(
                                    ps,
                                    xt_big[:, n * 8 + i, :],
                                    wt[:, i, :],
                                    start=(i == 0),
                                    stop=(i == 7),
                                )
                            nc.scalar.activation(
                                dst_big[:, n, jc * 512:(jc + 1) * 512],
                                ps, AF.Copy, scale=scale,
                            )

                # ---- scores: s_raw[b, n, m, h] = sum_d q[b,n,h,d] k[b,m,h,d]
                s_raw = sm_p.tile([128, NTOK, NTOK, H], F32, tag="sbuf4k")
                for n in range(NTOK):
                    for mq in range(4):
                        prod = sc_p.tile([128, 2, DIM], BF16, tag="sc")
                        nc.gpsimd.tensor_mul(
                            prod,
                            k_big[:, mq * 2:(mq + 1) * 2, :],
                            q_big[:, n:n + 1, :].broadcast_to([128, 2, DIM]),
                        )
                        nc.vector.tensor_reduce(
                            out=s_raw[:, n, mq * 2:(mq + 1) * 2, :],
                            in_=prod.rearrange("p m (h d) -> p m h d", d=HD),
                            axis=AX.X, op=ALU.add,
                        )

                # ---- talking-heads mix 1 (Wl, bl): s2[b, n, g, m]
                s2 = sm_p.tile([128, NTOK, H, NTOK], F32, tag="s2")
                for g in range(H):
                    dst = s2[:, :, g, :]
                    nc.vector.tensor_scalar(
                        out=dst, in0=s_raw[:, :, :, 0],
                        scalar1=float(Wl[0, g]), scalar2=float(bl[g]),
                        op0=ALU.mult, op1=ALU.add,
                    )
                    for h in range(1, H):
                        nc.vector.scalar_tensor_tensor(
                            out=dst, in0=s_raw[:, :, :, h],
                            scalar=float(Wl[h, g]), in1=dst,
                            op0=ALU.mult, op1=ALU.add,
                        )

                # ---- softmax over m (no max-subtraction: logits are O(1));
                # exp and renormalize run in place on s2
                nc.scalar.activation(
                    s2.rearrange("p a b c -> p (a b c)"),
                    s2.rearrange("p a b c -> p (a b c)"),
                    AF.Exp,
                )
                d_t = stat_p.tile([128, NTOK * H], F32, tag="dt")
                nc.vector.tensor_reduce(
                    out=d_t, in_=s2.rearrange("p a b c -> p (a b) c"),
                    axis=AX.X, op=ALU.add,
                )
                r_t = stat_p.tile([128, NTOK * H], F32, tag="rt")
                nc.vector.reciprocal(r_t, d_t)
                norm = s2
                nc.vector.tensor_mul(
                    norm.rearrange("p a b c -> p (a b) c"),
                    norm.rearrange("p a b c -> p (a b) c"),
                    r_t.unsqueeze(-1).broadcast_to([128, NTOK * H, NTOK]),
                )

                # ---- talking-heads mix 2 (Ww, bw): s3[b, n, g2, m]
                s3f = sm_p.tile([128, NTOK, H, NTOK], F32, tag="sbuf4k")
                for g2 in range(H):
                    dst = s3f[:, :, g2, :]
                    nc.vector.tensor_scalar(
                        out=dst, in0=norm[:, :, 0, :],
                        scalar1=float(Ww[0, g2]), scalar2=float(bw[g2]),
                        op0=ALU.mult, op1=ALU.add,
                    )
                    for g in range(1, H):
                        nc.vector.scalar_tensor_tensor(
                            out=dst, in0=norm[:, :, g, :],
                            scalar=float(Ww[g, g2]), in1=dst,
                            op0=ALU.mult, op1=ALU.add,
                        )
                s3 = sm_p.tile([128, NTOK, H, NTOK], BF16, tag="s3")
                nc.vector.tensor_copy(s3, s3f)

                # ---- AV + LayerNorm + Silu + A^T + output projection,
                # in two half-passes of 4 tokens to bound SBUF residency
                for half in range(2):
                    at_half = xa_p.tile([128, 4 * 32, 128], BF16, tag="xa")
                    for nn in range(4):
                        n = half * 4 + nn
                        o_t = o_p.tile([128, HID], BF16, tag="o")
                        for m in range(NTOK):
                            for ah in range(2):
                                oh = o_t[:, ah * 2048:(ah + 1) * 2048]
                                ohv = oh.rearrange("p (g e) -> p g e", g=H // 2)
                                coef = (
                                    s3[:, n, ah * 8:(ah + 1) * 8, m]
                                    .unsqueeze(-1)
                                    .broadcast_to([128, H // 2, GE])
                                )
                                vv = v_big[
                                    :, m, ah * 2048:(ah + 1) * 2048
                                ].rearrange("p (g e) -> p g e", g=H // 2)
                                if m == 0:
                                    nc.gpsimd.tensor_mul(ohv, vv, coef)
                                else:
                                    tmp = sc_p.tile([128, 2048], BF16, tag="sc")
                                    tv = tmp.rearrange("p (g e) -> p g e", g=H // 2)
                                    nc.gpsimd.tensor_mul(tv, vv, coef)
                                    nc.vector.tensor_add(oh, oh, tmp)

                        # LayerNorm stats
                        stats = stat_p.tile([128, 8, 6], F32, tag="bst")
                        ov8 = o_t.rearrange("p (s d) -> p s d", s=8)
                        for sg in range(8):
                            nc.vector.bn_stats(stats[:, sg, :], ov8[:, sg, :])
                        mv = stat_p.tile([128, 2], F32, tag="mv")
                        nc.vector.bn_aggr(mv, stats)
                        sd = stat_p.tile([128, 1], F32, tag="sd")
                        nc.scalar.activation(sd, mv[:, 1:2], AF.Sqrt, bias=eps_t)
                        rstd = stat_p.tile([128, 1], F32, tag="rstd")
                        nc.vector.reciprocal(rstd, sd)
                        nbias = stat_p.tile([128, 1], F32, tag="nb")
                        nc.vector.tensor_mul(nbias, mv[:, 0:1], rstd)
                        nc.vector.tensor_scalar_mul(nbias, nbias, -1.0)

                        # a = silu((o - mu) * rstd)   [gamma=1, beta=0 fast path]
                        a_t = a_p.tile([128, HID], BF16, tag="a")
                        if use_silu:
                            nc.scalar.activation(a_t, o_t, AF.Silu, bias=nbias, scale=rstd)
                        else:
                            nmu = stat_p.tile([128, 1], F32, tag="nmu")
                            nc.vector.tensor_scalar_mul(nmu, mv[:, 0:1], -1.0)
                            nc.scalar.activation(a_t, o_t, AF.Sigmoid, bias=nbias, scale=rstd)
                            ln_t = o_p.tile([128, HID], BF16, tag="ln")
                            nc.vector.tensor_scalar(
                                out=ln_t, in0=o_t, scalar1=nmu, scalar2=rstd,
                                op0=ALU.add, op1=ALU.mult,
                            )
                            nc.vector.tensor_mul(a_t, ln_t, a_t)

                        # A^T blocks for the output projection
                        for i in range(32):
                            ptr = psum_tr.tile([128, 128], BF16, tag="ptrb")
                            nc.tensor.transpose(ptr, a_t[:, i * 128:(i + 1) * 128], ident_bf)
                            nc.scalar.copy(at_half[:, nn * 32 + i, :], ptr)

                    # output projection for this half: out[b, n*1024+j] = a @ Wp
                    # Wp streamed in [128i x 8, 512] chunks; 4 psum chains (one
                    # per token) accumulate across the four i-subchunks.
                    for jc in range(2):
                        pss = []
                        for _pi in range(4):
                            ps_n = psum_mm.tile([128, 512], F32, tag="mm")
                            pss.append(ps_n)
                        for sub in range(4):
                            wpt = wst_p.tile([128, 8, 512], BF16, tag="w8")
                            for i8 in range(8):
                                i = sub * 8 + i8
                                nc.sync.dma_start(
                                    out=wpt[:, i8, :],
                                    in_=wp_d[i * 128:(i + 1) * 128, jc * 512:(jc + 1) * 512],
                                )
                            for nn in range(4):
                                for i8 in range(8):
                                    i = sub * 8 + i8
                                    nc.tensor.matmul(
                                        pss[nn],
                                        at_half[:, nn * 32 + i, :],
                                        wpt[:, i8, :],
                                        start=(sub == 0 and i8 == 0),
                                        stop=(sub == 3 and i8 == 7),
                                    )
                        for nn in range(4):
                            n = half * 4 + nn
                            osb = outsb_p.tile([128, 512], F32, tag="osb")
                            nc.scalar.copy(osb, pss[nn])
                            nc.sync.dma_start(
                                out=out_d[b0:b0 + 128, n * DIM + jc * 512:n * DIM + (jc + 1) * 512],
                                in_=osb,
                            )
    import bass_rust as _bass_rust
    _bass_rust.move_matmul_waits_to_ldweights(nc.m)
    _bass_rust.generate_event_semaphores(nc)
    return nc


def _to_bf16(a):
    return np.asarray(a, dtype=np.float32).astype(ml_dtypes.bfloat16)


def kernel(**inputs) -> np.ndarray:
    global LAST_RESULT
    x = np.ascontiguousarray(np.asarray(inputs["x"], dtype=np.float32))
    Wl = np.asarray(inputs["Wl"], np.float32)
    Ww = np.asarray(inputs["Ww"], np.float32)
    bl = np.asarray(inputs["bl"], np.float32)
    bw = np.asarray(inputs["bw"], np.float32)

    Wq = np.asarray(inputs["Wq"], np.float32)
    Wk = np.asarray(inputs["Wk"], np.float32)
    Wv = np.asarray(inputs["Wv"], np.float32)
    Wp = np.asarray(inputs["Wp"], np.float32)
    gamma = np.asarray(inputs["gamma"], np.float32)
    beta = np.asarray(inputs["beta"], np.float32)
    for name in ("bq", "bk", "bv", "bp"):
        assert not np.any(np.asarray(inputs[name], np.float32)), f"{name} != 0 unsupported"
    assert np.all(gamma == 1.0) and not np.any(beta), "non-identity LN unsupported"

    nc = build_program(Wl, Ww, bl, bw)

    wq = _to_bf16(Wq)
    wk = _to_bf16(Wk)
    wv = _to_bf16(Wv)
    wp = _to_bf16(Wp)
    in_maps = [
        {
            "x": x[c * BPC:(c + 1) * BPC],
            "wq": wq, "wk": wk, "wv": wv, "wp": wp,
        }
        for c in range(N_CORES)
    ]
    res = run_bass_kernel_spmd(nc, in_maps, list(range(N_CORES)))
    LAST_RESULT = res
    if os.environ.get("BASS_BENCH"):
        import time as _time
        global LAST_TIMES
        LAST_TIMES = []
        for _ in range(int(os.environ.get("BASS_BENCH_REPEATS", "3"))):
            t0 = _time.time()
            run_bass_kernel_spmd(nc, in_maps, list(range(N_CORES)))
            LAST_TIMES.append(_time.time() - t0)
    out = np.concatenate(
        [np.asarray(res.results[c]["out"]) for c in range(N_CORES)], axis=0
    ).astype(np.float32)
    return out


# revision 27
# speedup vs baseline: 1.2111x; 1.2111x over previous
import os
import sys

import numpy as np

sys.path.insert(0, "/opt/trn_rl_repo")

import ml_dtypes  # noqa: E402
from contextlib import ExitStack  # noqa: E402

import concourse.bass as bass  # noqa: E402
import concourse.tile as tile  # noqa: E402
from concourse import mybir  # noqa: E402
from concourse.bass_utils import run_bass_kernel_spmd  # noqa: E402
from concourse.kernels.tile_matmul import make_identity  # noqa: E402

AF = mybir.ActivationFunctionType
ALU = mybir.AluOpType
AX = mybir.AxisListType
F32 = mybir.dt.float32
BF16 = mybir.dt.bfloat16

N_CORES = 8
B_FULL = 2048
BPC = B_FULL // N_CORES      # 256 batch rows per core
NTOK = 8
DIM = 1024
HID = 4096
H = 16                       # heads
HD = DIM // H                # 64 head dim
GE = HID // H                # 256 v-dim per head
SCALE = HD ** -0.5
LN_EPS = 1e-5

LAST_RESULT = None
LAST_TIMES = None


def build_program(nbt=BPC // 128, use_silu=True):
    """Per-core SPMD program. Token order within a 128-row b-tile is n-major:
    GEMM output tiles are [128 b, ...] for a fixed token n, which is the
    layout the attention middle needs (batch in partitions).

    The talking-heads mixes + softmax sums run on the TensorEngine in a
    transposed [(head, m), b] layout against host-built block-diagonal
    matrices (dram param "wm"); AV runs as PE matmuls with diagonal
    coefficient matrices accumulating over m in PSUM.
    """
    bpc = nbt * 128
    nc = bass.Bass("TRN2", target_bir_lowering=False, debug=False)
    x_d = nc.declare_dram_parameter("x", [bpc, NTOK * DIM], BF16, isOutput=False)
    wq_d = nc.declare_dram_parameter("wq", [DIM, DIM], BF16, isOutput=False)
    wk_d = nc.declare_dram_parameter("wk", [DIM, DIM], BF16, isOutput=False)
    wv_d = nc.declare_dram_parameter("wv", [DIM, HID], BF16, isOutput=False)
    wp_d = nc.declare_dram_parameter("wp", [HID, DIM], BF16, isOutput=False)
    # packed mix consts: [m1 | m2 | onesD] along the free dim
    wm_d = nc.declare_dram_parameter("wm", [128, 272], BF16, isOutput=False)
    wb_d = nc.declare_dram_parameter("wb", [128, 2], F32, isOutput=False)
    out_d = nc.declare_dram_parameter("out", [bpc, NTOK * DIM], F32, isOutput=True)

    with tile.TileContext(nc) as tc:
        with ExitStack() as ctx:
            ep = ctx.enter_context
            const_p = ep(tc.tile_pool(name="const", bufs=1))
            xa_p = ep(tc.tile_pool(name="xa", bufs=1))     # A^T halves
            xt_p = ep(tc.tile_pool(name="xt", bufs=1))     # X^T
            wst_p = ep(tc.tile_pool(name="wst", bufs=2))   # weight stream chunks
            q_p = ep(tc.tile_pool(name="q", bufs=1))
            k_p = ep(tc.tile_pool(name="k", bufs=1))
            v_p = ep(tc.tile_pool(name="v", bufs=1))
            sc_p = ep(tc.tile_pool(name="sc", bufs=2))     # scores scratch
            sm_p = ep(tc.tile_pool(name="sm", bufs=1))     # s_raw / s3b
            tsm_p = ep(tc.tile_pool(name="tsm", bufs=2))   # small transposed tiles
            dg_p = ep(tc.tile_pool(name="dg", bufs=6))     # diag coef tiles
            o_p = ep(tc.tile_pool(name="o", bufs=1))
            a_p = ep(tc.tile_pool(name="a", bufs=2))
            outsb_p = ep(tc.tile_pool(name="outsb", bufs=2))
            stat_p = ep(tc.tile_pool(name="stat", bufs=2))
            psum_mm = ep(tc.tile_pool(name="psum_mm", bufs=4, space="PSUM"))
            psum_av = ep(tc.tile_pool(name="psum_av", bufs=2, space="PSUM"))
            psum_tr = ep(tc.tile_pool(name="psum_tr", bufs=1, space="PSUM"))

            ident_f32 = const_p.tile([128, 128], F32)
            make_identity(nc, ident_f32)
            ident_bf = const_p.tile([128, 128], BF16)
            make_identity(nc, ident_bf)
            eps_t = const_p.tile([128, 1], F32)
            nc.vector.memset(eps_t, LN_EPS)
            wm_sb = const_p.tile([128, 272], BF16)
            nc.sync.dma_start(out=wm_sb, in_=wm_d[:, :])
            m1_sb = wm_sb[:, 0:128]
            m2_sb = wm_sb[:, 128:256]
            onesd_sb = wm_sb[:, 256:272]
            wb_sb = const_p.tile([128, 2], F32)
            nc.sync.dma_start(out=wb_sb, in_=wb_d[:, :])
            bl_col = wb_sb[:, 0:1]
            bw_col = wb_sb[:, 1:2]

            for bt in range(nbt):
                b0 = bt * 128

                # ---- X^T: transpose x rows into [dim, batch] blocks per token
                xt_big = xt_p.tile([128, NTOK * 8, 128], BF16, tag="xt")
                for n in range(NTOK):
                    for i in range(8):
                        nc.sync.dma_start_transpose(
                            out=xt_big[:, n * 8 + i, :],
                            in_=x_d[b0:b0 + 128, n * DIM + i * 128:n * DIM + (i + 1) * 128],
                        )

                # ---- QKV GEMMs (weights streamed in 512-wide chunks)
                q_big = q_p.tile([128, NTOK, DIM], BF16, tag="q")
                k_big = k_p.tile([128, NTOK, DIM], BF16, tag="k")
                v_big = v_p.tile([128, NTOK, HID], BF16, tag="v")
                gemms = [
                    (wq_d, DIM // 512, q_big, SCALE),
                    (wk_d, DIM // 512, k_big, 1.0),
                    (wv_d, HID // 512, v_big, 1.0),
                ]
                for w_d, njc, dst_big, scale in gemms:
                    for jc in range(njc):
                        wt = wst_p.tile([128, 8, 512], BF16, tag="w8")
                        for i in range(8):
                            nc.sync.dma_start(
                                out=wt[:, i, :],
                                in_=w_d[i * 128:(i + 1) * 128, jc * 512:(jc + 1) * 512],
                            )
                        for n in range(NTOK):
                            ps = psum_mm.tile([128, 512], F32, tag="mm")
                            for i in range(8):
                                nc.tensor.matmul(
                                    ps,
                                    xt_big[:, n * 8 + i, :],
                                    wt[:, i, :],
                                    start=(i == 0),
                                    stop=(i == 7),
                                )
                            nc.scalar.activation(
                                dst_big[:, n, jc * 512:(jc + 1) * 512],
                                ps, AF.Copy, scale=scale,
                            )

                # ---- scores: s_raw[b, n, (h, m)] = sum_d q[b,n,h,d] k[b,m,h,d]
                s_raw = sm_p.tile([128, NTOK, H, NTOK], F32, tag="sraw")
                for n in range(NTOK):
                    sr_mh = s_raw[:, n].rearrange("p h m -> p m h")
                    for mq in range(4):
                        prod = sc_p.tile([128, 2, DIM], BF16, tag="sc")
                        nc.gpsimd.tensor_mul(
                            prod,
                            k_big[:, mq * 2:(mq + 1) * 2, :],
                            q_big[:, n:n + 1, :].broadcast_to([128, 2, DIM]),
                        )
                        nc.vector.tensor_reduce(
                            out=sr_mh[:, mq * 2:(mq + 1) * 2, :],
                            in_=prod.rearrange("p m (h d) -> p m h d", d=HD),
                            axis=AX.X, op=ALU.add,
                        )

                # ---- attention middle, per token n, in transposed
                # [(head, m), b] space on the TensorEngine
                s3b_all = sm_p.tile([128, NTOK, H, NTOK], F32, tag="s3b")
                for n in range(NTOK):
                    # transpose scores to [(h, m), b]
                    ptr1 = psum_tr.tile([128, 128], F32, tag="ptr")
                    nc.tensor.transpose(
                        ptr1, s_raw[:, n].rearrange("p h m -> p (h m)"), ident_f32
                    )
                    srT = tsm_p.tile([128, 128], BF16, tag="srT")
                    nc.scalar.copy(srT, ptr1)
                    # talking-heads mix 1 + bias + exp (no max-subtraction:
                    # logits are O(1) for this problem's data)
                    psE = psum_tr.tile([128, 128], F32, tag="ptr")
                    nc.tensor.matmul(psE, m1_sb, srT, start=True, stop=True)
                    e_t = tsm_p.tile([128, 128], BF16, tag="et")
                    nc.scalar.activation(e_t, psE, AF.Exp, bias=bl_col)
                    # softmax denominators per (g, b), expanded back to rows
                    psD = psum_tr.tile([16, 128], F32, tag="ptr")
                    nc.tensor.matmul(psD, onesd_sb, e_t, start=True, stop=True)
                    rd_t = tsm_p.tile([16, 128], F32, tag="rd")
                    nc.vector.reciprocal(rd_t, psD)
                    rdx = tsm_p.tile([128, 128], F32, tag="rdx")
                    rd_bc = bass.AP(
                        tensor=rd_t.tensor,
                        offset=rd_t.offset,
                        ap=[rd_t.ap[0], [0, 8], rd_t.ap[1]],
                    )
                    nc.sync.dma_start(out=rdx, in_=rd_bc)
                    en_t = tsm_p.tile([128, 128], BF16, tag="en")
                    nc.vector.tensor_mul(en_t, e_t, rdx)
                    # talking-heads mix 2 + bias, then transpose back to b-major
                    psS3 = psum_tr.tile([128, 128], F32, tag="ptr")
                    nc.tensor.matmul(psS3, m2_sb, en_t, start=True, stop=True)
                    s3T = tsm_p.tile([128, 128], BF16, tag="s3T")
                    nc.scalar.activation(s3T, psS3, AF.Identity, bias=bw_col)
                    ptr2 = psum_tr.tile([128, 128], BF16, tag="ptrb")
                    nc.tensor.transpose(ptr2, s3T, ident_bf)
                    nc.scalar.copy(s3b_all[:, n].rearrange("p g m -> p (g m)"), ptr2)

                # ---- AV on PE: diag(s3) @ V slices, accumulated over m in
                # PSUM; then LayerNorm + Silu + A^T + output projection
                for half in range(4):
                    at_half = xa_p.tile([128, 2 * 32, 128], BF16, tag="xa")
                    for nn in range(2):
                        n = half * 2 + nn
                        o_t = o_p.tile([128, HID], BF16, tag="o")
                        GSPLIT = 7
                        wid = GSPLIT * GE
                        oslice = o_t[:, 0:wid].rearrange("p (g e) -> p g e", g=GSPLIT)
                        for m in range(NTOK):
                            coef = (
                                s3b_all[:, n, 0:GSPLIT, m]
                                .unsqueeze(-1)
                                .broadcast_to([128, GSPLIT, GE])
                            )
                            vv = v_big[:, m, 0:wid].rearrange(
                                "p (g e) -> p g e", g=GSPLIT
                            )
                            if m == 0:
                                nc.gpsimd.tensor_mul(oslice, vv, coef)
                            else:
                                tmp = sc_p.tile([128, GSPLIT * GE], BF16, tag="sc")
                                tv = tmp.rearrange("p (g e) -> p g e", g=GSPLIT)
                                nc.gpsimd.tensor_mul(tv, vv, coef)
                                nc.vector.tensor_add(
                                    o_t[:, 0:wid], o_t[:, 0:wid], tmp
                                )
                        for g in range(GSPLIT, H):
                            psO = psum_av.tile([128, GE], F32, tag="av")
                            for m in range(NTOK):
                                dg_t = dg_p.tile([128, 128], BF16, tag="dg")
                                eng = nc.vector if (m % 2 == 0) else nc.gpsimd
                                eng.tensor_scalar_mul(
                                    dg_t, ident_bf, s3b_all[:, n, g, m:m + 1]
                                )
                                nc.tensor.matmul(
                                    psO, dg_t,
                                    v_big[:, m, g * GE:(g + 1) * GE],
                                    start=(m == 0), stop=(m == 7),
                                )
                            nc.scalar.copy(o_t[:, g * GE:(g + 1) * GE], psO)

                        # LayerNorm stats
                        stats = stat_p.tile([128, 8, 6], F32, tag="bst")
                        ov8 = o_t.rearrange("p (s d) -> p s d", s=8)
                        for sg in range(8):
                            nc.vector.bn_stats(stats[:, sg, :], ov8[:, sg, :])
                        mv = stat_p.tile([128, 2], F32, tag="mv")
                        nc.vector.bn_aggr(mv, stats)
                        sd = stat_p.tile([128, 1], F32, tag="sd")
                        nc.scalar.activation(sd, mv[:, 1:2], AF.Sqrt, bias=eps_t)
                        rstd = stat_p.tile([128, 1], F32, tag="rstd")
                        nc.vector.reciprocal(rstd, sd)
                        nbias = stat_p.tile([128, 1], F32, tag="nb")
                        nc.vector.tensor_mul(nbias, mv[:, 0:1], rstd)
                        nc.vector.tensor_scalar_mul(nbias, nbias, -1.0)

                        # a = silu((o - mu) * rstd)   [gamma=1, beta=0 fast path]
                        a_t = a_p.tile([128, HID], BF16, tag="a")
                        if use_silu:
                            nc.scalar.activation(a_t, o_t, AF.Silu, bias=nbias, scale=rstd)
                        else:
                            nmu = stat_p.tile([128, 1], F32, tag="nmu")
                            nc.vector.tensor_scalar_mul(nmu, mv[:, 0:1], -1.0)
                            nc.scalar.activation(a_t, o_t, AF.Sigmoid, bias=nbias, scale=rstd)
                            ln_t = o_p.tile([128, HID], BF16, tag="ln")
                            nc.vector.tensor_scalar(
                                out=ln_t, in0=o_t, scalar1=nmu, scalar2=rstd,
                                op0=ALU.add, op1=ALU.mult,
                            )
                            nc.vector.tensor_mul(a_t, ln_t, a_t)

                        # A^T blocks for the output projection
                        for i in range(32):
                            ptr = psum_tr.tile([128, 128], BF16, tag="ptrb")
                            nc.tensor.transpose(ptr, a_t[:, i * 128:(i + 1) * 128], ident_bf)
                            nc.scalar.copy(at_half[:, nn * 32 + i, :], ptr)

                    # output projection for this half: out[b, n*1024+j] = a @ Wp
                    for jc in range(2):
                        pss = []
                        for _pi in range(2):
                            ps_n = psum_mm.tile([128, 512], F32, tag="mm")
                            pss.append(ps_n)
                        for sub in range(4):
                            wpt = wst_p.tile([128, 8, 512], BF16, tag="w8")
                            for i8 in range(8):
                                i = sub * 8 + i8
                                nc.sync.dma_start(
                                    out=wpt[:, i8, :],
                                    in_=wp_d[i * 128:(i + 1) * 128, jc * 512:(jc + 1) * 512],
                                )
                            for nn in range(2):
                                for i8 in range(8):
                                    i = sub * 8 + i8
                                    nc.tensor.matmul(
                                        pss[nn],
                                        at_half[:, nn * 32 + i, :],
                                        wpt[:, i8, :],
                                        start=(sub == 0 and i8 == 0),
                                        stop=(sub == 3 and i8 == 7),
                                    )
                        for nn in range(2):
                            n = half * 2 + nn
                            osb = outsb_p.tile([128, 512], F32, tag="osb")
                            nc.scalar.copy(osb, pss[nn])
                            nc.sync.dma_start(
                                out=out_d[b0:b0 + 128, n * DIM + jc * 512:n * DIM + (jc + 1) * 512],
                                in_=osb,
                            )
    import bass_rust as _bass_rust
    _bass_rust.move_matmul_waits_to_ldweights(nc.m)
    _bass_rust.generate_event_semaphores(nc)
    return nc


def build_mix_consts(Wl, Ww, bl, bw):
    """Host-built block-diagonal mix matrices for the transposed
    [(head, m), b] attention space. Row/col order is head-major: r = g*8+m."""
    m1 = np.zeros((128, 128), np.float32)   # [(h,m), (g,m)] = Wl[h,g]
    m2 = np.zeros((128, 128), np.float32)   # [(g,m), (g2,m)] = Ww[g,g2]
    for m in range(NTOK):
        for h in range(H):
            for g in range(H):
                m1[h * 8 + m, g * 8 + m] = Wl[h, g]
                m2[h * 8 + m, g * 8 + m] = Ww[h, g]
    onesd = np.zeros((128, 16), np.float32)  # [(g,m), g'] = (g == g')
    for g in range(H):
        for m in range(NTOK):
            onesd[g * 8 + m, g] = 1.0
    wm = np.concatenate([m1, m2, onesd], axis=1).astype(ml_dtypes.bfloat16)
    wb = np.zeros((128, 2), np.float32)
    for g in range(H):
        for m in range(NTOK):
            wb[g * 8 + m, 0] = bl[g]
            wb[g * 8 + m, 1] = bw[g]
    return wm, wb


def _to_bf16(a):
    return np.asarray(a, dtype=np.float32).astype(ml_dtypes.bfloat16)


def kernel(**inputs) -> np.ndarray:
    global LAST_RESULT, LAST_TIMES
    x = np.ascontiguousarray(np.asarray(inputs["x"], dtype=np.float32))
    Wl = np.asarray(inputs["Wl"], np.float32)
    Ww = np.asarray(inputs["Ww"], np.float32)
    bl = np.asarray(inputs["bl"], np.float32)
    bw = np.asarray(inputs["bw"], np.float32)

    gamma = np.asarray(inputs["gamma"], np.float32)
    beta = np.asarray(inputs["beta"], np.float32)
    for name in ("bq", "bk", "bv", "bp"):
        assert not np.any(np.asarray(inputs[name], np.float32)), f"{name} != 0 unsupported"
    assert np.all(gamma == 1.0) and not np.any(beta), "non-identity LN unsupported"

    nc = build_program()
    wm, wb = build_mix_consts(Wl, Ww, bl, bw)
    xb16 = x.astype(ml_dtypes.bfloat16)

    in_maps = [
        {
            "x": xb16[c * BPC:(c + 1) * BPC],
            "wq": _to_bf16(inputs["Wq"]),
            "wk": _to_bf16(inputs["Wk"]),
            "wv": _to_bf16(inputs["Wv"]),
            "wp": _to_bf16(inputs["Wp"]),
            "wm": wm,
            "wb": wb,
        }
        for c in range(N_CORES)
    ]
    res = run_bass_kernel_spmd(nc, in_maps, list(range(N_CORES)))
    LAST_RESULT = res
    if os.environ.get("BASS_BENCH"):
        import time as _time
        LAST_TIMES = []
        for _ in range(int(os.environ.get("BASS_BENCH_REPEATS", "3"))):
            t0 = _time.time()
            run_bass_kernel_spmd(nc, in_maps, list(range(N_CORES)))
            LAST_TIMES.append(_time.time() - t0)
    out = np.concatenate(
        [np.asarray(res.results[c]["out"]) for c in range(N_CORES)], axis=0
    ).astype(np.float32)
    return out


# revision 32
# speedup vs baseline: 1.2855x; 1.0614x over previous
import os
import sys

import numpy as np

sys.path.insert(0, "/opt/trn_rl_repo")

import ml_dtypes  # noqa: E402
from contextlib import ExitStack  # noqa: E402

import concourse.bass as bass  # noqa: E402
import concourse.tile as tile  # noqa: E402
from concourse import mybir  # noqa: E402
from concourse.bass_utils import run_bass_kernel_spmd  # noqa: E402
from concourse.kernels.tile_matmul import make_identity  # noqa: E402

AF = mybir.ActivationFunctionType
ALU = mybir.AluOpType
AX = mybir.AxisListType
F32 = mybir.dt.float32
BF16 = mybir.dt.bfloat16

N_CORES = 8
B_FULL = 2048
BPC = B_FULL // N_CORES      # 256 batch rows per core
NTOK = 8
DIM = 1024
HID = 4096
H = 16                       # heads
HD = DIM // H                # 64 head dim
GE = HID // H                # 256 v-dim per head
SCALE = HD ** -0.5
LN_EPS = 1e-5

LAST_RESULT = None
LAST_TIMES = None


def build_program(nbt=BPC // 128, use_silu=True):
    """Per-core SPMD program. Token order within a 128-row b-tile is n-major:
    GEMM output tiles are [128 b, ...] for a fixed token n, which is the
    layout the attention middle needs (batch in partitions).

    The talking-heads mixes + softmax sums run on the TensorEngine in a
    transposed [(head, m), b] layout against host-built block-diagonal
    matrices (dram param "wm"); AV runs as PE matmuls with diagonal
    coefficient matrices accumulating over m in PSUM.
    """
    bpc = nbt * 128
    nc = bass.Bass("TRN2", target_bir_lowering=False, debug=False)
    x_d = nc.declare_dram_parameter("x", [bpc, NTOK * DIM], BF16, isOutput=False)
    wq_d = nc.declare_dram_parameter("wq", [DIM, DIM], BF16, isOutput=False)
    wk_d = nc.declare_dram_parameter("wk", [DIM, DIM], BF16, isOutput=False)
    wv_d = nc.declare_dram_parameter("wv", [DIM, HID], BF16, isOutput=False)
    wp_d = nc.declare_dram_parameter("wp", [HID, DIM], BF16, isOutput=False)
    # packed mix consts: [m1 | m2 | onesD] along the free dim
    wm_d = nc.declare_dram_parameter("wm", [128, 272], BF16, isOutput=False)
    wb_d = nc.declare_dram_parameter("wb", [128, 2], F32, isOutput=False)
    out_d = nc.declare_dram_parameter("out", [bpc, NTOK * DIM], F32, isOutput=True)

    with tile.TileContext(nc) as tc:
        with ExitStack() as ctx:
            ep = ctx.enter_context
            const_p = ep(tc.tile_pool(name="const", bufs=1))
            xa_p = ep(tc.tile_pool(name="xa", bufs=2))     # A^T quarters
            xt_p = ep(tc.tile_pool(name="xt", bufs=1))     # X^T
            wst_p = ep(tc.tile_pool(name="wst", bufs=2))   # weight stream chunks
            q_p = ep(tc.tile_pool(name="q", bufs=8))
            k_p = ep(tc.tile_pool(name="k", bufs=1))
            v_p = ep(tc.tile_pool(name="v", bufs=1))
            sc_p = ep(tc.tile_pool(name="sc", bufs=2))     # scores scratch
            sm_p = ep(tc.tile_pool(name="sm", bufs=1))     # s_raw / s3b
            tsm_p = ep(tc.tile_pool(name="tsm", bufs=2))   # small transposed tiles
            dg_p = ep(tc.tile_pool(name="dg", bufs=5))     # diag coef tiles
            o_p = ep(tc.tile_pool(name="o", bufs=1))
            a_p = ep(tc.tile_pool(name="a", bufs=1))
            outsb_p = ep(tc.tile_pool(name="outsb", bufs=1))
            stat_p = ep(tc.tile_pool(name="stat", bufs=1))
            psum_mm = ep(tc.tile_pool(name="psum_mm", bufs=4, space="PSUM"))
            psum_av = ep(tc.tile_pool(name="psum_av", bufs=2, space="PSUM"))
            psum_tr = ep(tc.tile_pool(name="psum_tr", bufs=1, space="PSUM"))

            ident_f32 = const_p.tile([128, 128], F32)
            make_identity(nc, ident_f32)
            ident_bf = const_p.tile([128, 128], BF16)
            make_identity(nc, ident_bf)
            eps_t = const_p.tile([128, 1], F32)
            nc.vector.memset(eps_t, LN_EPS)
            wm_sb = const_p.tile([128, 272], BF16)
            nc.sync.dma_start(out=wm_sb, in_=wm_d[:, :])
            m1_sb = wm_sb[:, 0:128]
            m2_sb = wm_sb[:, 128:256]
            onesd_sb = wm_sb[:, 256:272]
            wb_sb = const_p.tile([128, 2], F32)
            nc.sync.dma_start(out=wb_sb, in_=wb_d[:, :])
            bl_col = wb_sb[:, 0:1]
            bw_col = wb_sb[:, 1:2]

            for bt in range(nbt):
                b0 = bt * 128

                # ---- X^T: transpose x rows into [dim, batch] blocks per token
                xt_big = xt_p.tile([128, NTOK * 8, 128], BF16, tag="xt")
                for n in range(NTOK):
                    for i in range(8):
                        nc.sync.dma_start_transpose(
                            out=xt_big[:, n * 8 + i, :],
                            in_=x_d[b0:b0 + 128, n * DIM + i * 128:n * DIM + (i + 1) * 128],
                        )

                # ---- QKV GEMMs (weights streamed in 512-wide chunks)
                q_tiles = []
                for _qi in range(NTOK):
                    q_n = q_p.tile([128, DIM], BF16, tag="q")
                    q_tiles.append(q_n)
                k_big = k_p.tile([128, NTOK, DIM], BF16, tag="k")
                v_big = v_p.tile([128, NTOK, HID], BF16, tag="v")
                gemms = [
                    (wq_d, DIM // 512, None, SCALE),
                    (wk_d, DIM // 512, k_big, 1.0),
                    (wv_d, HID // 512, v_big, 1.0),
                ]
                for w_d, njc, dst_big, scale in gemms:
                    for jc in range(njc):
                        wt = wst_p.tile([128, 8, 512], BF16, tag="w8")
                        for i in range(8):
                            nc.sync.dma_start(
                                out=wt[:, i, :],
                                in_=w_d[i * 128:(i + 1) * 128, jc * 512:(jc + 1) * 512],
                            )
                        for n in range(NTOK):
                            ps = psum_mm.tile([128, 512], F32, tag="mm")
                            for i in range(8):
                                nc.tensor.matmul(
                                    ps,
                                    xt_big[:, n * 8 + i, :],
                                    wt[:, i, :],
                                    start=(i == 0),
                                    stop=(i == 7),
                                )
                            dst_ap = (
                                q_tiles[n][:, jc * 512:(jc + 1) * 512]
                                if dst_big is None
                                else dst_big[:, n, jc * 512:(jc + 1) * 512]
                            )
                            nc.scalar.activation(dst_ap, ps, AF.Copy, scale=scale)

                # ---- scores: s_raw[b, n, (h, m)] = sum_d q[b,n,h,d] k[b,m,h,d]
                s_raw = sm_p.tile([128, NTOK, H, NTOK], BF16, tag="sraw")
                for n in range(NTOK):
                    sr_mh = s_raw[:, n].rearrange("p h m -> p m h")
                    for mq in range(4):
                        prod = sc_p.tile([128, 2, DIM], BF16, tag="sc")
                        nc.gpsimd.tensor_mul(
                            prod,
                            k_big[:, mq * 2:(mq + 1) * 2, :],
                            q_tiles[n].unsqueeze(1).broadcast_to([128, 2, DIM]),
                        )
                        with nc.allow_low_precision("bf16 scores are well within tolerance"):
                            nc.vector.tensor_reduce(
                                out=sr_mh[:, mq * 2:(mq + 1) * 2, :],
                                in_=prod.rearrange("p m (h d) -> p m h d", d=HD),
                                axis=AX.X, op=ALU.add,
                            )

                # ---- attention middle, per token n, in transposed
                # [(head, m), b] space on the TensorEngine
                s3b_all = sm_p.tile([128, NTOK, H, NTOK], F32, tag="s3b")
                for n in range(NTOK):
                    # transpose scores to [(h, m), b]
                    ptr1 = psum_tr.tile([128, 128], BF16, tag="ptrb")
                    nc.tensor.transpose(
                        ptr1, s_raw[:, n].rearrange("p h m -> p (h m)"), ident_bf
                    )
                    srT = tsm_p.tile([128, 128], BF16, tag="srT")
                    nc.scalar.copy(srT, ptr1)
                    # talking-heads mix 1 + bias + exp (no max-subtraction:
                    # logits are O(1) for this problem's data)
                    psE = psum_tr.tile([128, 128], F32, tag="ptr")
                    nc.tensor.matmul(psE, m1_sb, srT, start=True, stop=True)
                    e_t = tsm_p.tile([128, 128], BF16, tag="et")
                    nc.scalar.activation(e_t, psE, AF.Exp, bias=bl_col)
                    # softmax denominators per (g, b), expanded back to rows
                    psD = psum_tr.tile([16, 128], F32, tag="ptr")
                    nc.tensor.matmul(psD, onesd_sb, e_t, start=True, stop=True)
                    rd_t = tsm_p.tile([16, 128], F32, tag="rd")
                    nc.vector.reciprocal(rd_t, psD)
                    rdx = tsm_p.tile([128, 128], F32, tag="rdx")
                    rd_bc = bass.AP(
                        tensor=rd_t.tensor,
                        offset=rd_t.offset,
                        ap=[rd_t.ap[0], [0, 8], rd_t.ap[1]],
                    )
                    nc.sync.dma_start(out=rdx, in_=rd_bc)
                    en_t = tsm_p.tile([128, 128], BF16, tag="en")
                    nc.vector.tensor_mul(en_t, e_t, rdx)
                    # talking-heads mix 2 + bias, then transpose back to b-major
                    psS3 = psum_tr.tile([128, 128], F32, tag="ptr")
                    nc.tensor.matmul(psS3, m2_sb, en_t, start=True, stop=True)
                    s3T = tsm_p.tile([128, 128], BF16, tag="s3T")
                    nc.scalar.activation(s3T, psS3, AF.Identity, bias=bw_col)
                    ptr2 = psum_tr.tile([128, 128], BF16, tag="ptrb")
                    nc.tensor.transpose(ptr2, s3T, ident_bf)
                    nc.scalar.copy(s3b_all[:, n].rearrange("p g m -> p (g m)"), ptr2)

                # ---- AV on PE: diag(s3) @ V slices, accumulated over m in
                # PSUM; then LayerNorm + Silu + A^T + output projection
                for half in range(4):
                    at_half = xa_p.tile([128, 2 * 32, 128], BF16, tag="xa")
                    for nn in range(2):
                        n = half * 2 + nn
                        o_t = o_p.tile([128, HID], BF16, tag="o")
                        GSPLIT = 7
                        wid = GSPLIT * GE
                        oslice = o_t[:, 0:wid].rearrange("p (g e) -> p g e", g=GSPLIT)
                        for m in range(NTOK):
                            for gh, g1 in ((0, 4), (4, GSPLIT)):
                                ge0, ge1 = gh * GE, g1 * GE
                                coef = (
                                    s3b_all[:, n, gh:g1, m]
                                    .unsqueeze(-1)
                                    .broadcast_to([128, g1 - gh, GE])
                                )
                                vv = v_big[:, m, ge0:ge1].rearrange(
                                    "p (g e) -> p g e", g=g1 - gh
                                )
                                if m == 0:
                                    nc.gpsimd.tensor_mul(
                                        oslice[:, gh:g1], vv, coef
                                    )
                                else:
                                    tmp = sc_p.tile([128, 4 * GE], BF16, tag="sc")
                                    tv = tmp[:, 0:(g1 - gh) * GE].rearrange(
                                        "p (g e) -> p g e", g=g1 - gh
                                    )
                                    nc.gpsimd.tensor_mul(tv, vv, coef)
                                    nc.vector.tensor_add(
                                        o_t[:, ge0:ge1], o_t[:, ge0:ge1],
                                        tmp[:, 0:(g1 - gh) * GE],
                                    )
                        for g in range(GSPLIT, H):
                            psO = psum_av.tile([128, GE], F32, tag="av")
                            for m in range(NTOK):
                                dg_t = dg_p.tile([128, 128], BF16, tag="dg")
                                eng = nc.vector if (m % 2 == 0) else nc.gpsimd
                                eng.tensor_scalar_mul(
                                    dg_t, ident_bf, s3b_all[:, n, g, m:m + 1]
                                )
                                nc.tensor.matmul(
                                    psO, dg_t,
                                    v_big[:, m, g * GE:(g + 1) * GE],
                                    start=(m == 0), stop=(m == 7),
                                )
                            nc.scalar.copy(o_t[:, g * GE:(g + 1) * GE], psO)

                        # LayerNorm stats
                        stats = stat_p.tile([128, 8, 6], F32, tag="bst")
                        ov8 = o_t.rearrange("p (s d) -> p s d", s=8)
                        for sg in range(8):
                            nc.vector.bn_stats(stats[:, sg, :], ov8[:, sg, :])
                        mv = stat_p.tile([128, 2], F32, tag="mv")
                        nc.vector.bn_aggr(mv, stats)
                        sd = stat_p.tile([128, 1], F32, tag="sd")
                        nc.scalar.activation(sd, mv[:, 1:2], AF.Sqrt, bias=eps_t)
                        rstd = stat_p.tile([128, 1], F32, tag="rstd")
                        nc.vector.reciprocal(rstd, sd)
                        nbias = stat_p.tile([128, 1], F32, tag="nb")
                        nc.vector.tensor_mul(nbias, mv[:, 0:1], rstd)
                        nc.vector.tensor_scalar_mul(nbias, nbias, -1.0)

                        # a = silu((o - mu) * rstd)   [gamma=1, beta=0 fast path]
                        a_t = a_p.tile([128, HID], BF16, tag="a")
                        if use_silu:
                            nc.scalar.activation(a_t, o_t, AF.Silu, bias=nbias, scale=rstd)
                        else:
                            nmu = stat_p.tile([128, 1], F32, tag="nmu")
                            nc.vector.tensor_scalar_mul(nmu, mv[:, 0:1], -1.0)
                            nc.scalar.activation(a_t, o_t, AF.Sigmoid, bias=nbias, scale=rstd)
                            ln_t = o_p.tile([128, HID], BF16, tag="ln")
                            nc.vector.tensor_scalar(
                                out=ln_t, in0=o_t, scalar1=nmu, scalar2=rstd,
                                op0=ALU.add, op1=ALU.mult,
                            )
                            nc.vector.tensor_mul(a_t, ln_t, a_t)

                        # A^T blocks for the output projection
                        for i in range(32):
                            ptr = psum_tr.tile([128, 128], BF16, tag="ptrb")
                            nc.tensor.transpose(ptr, a_t[:, i * 128:(i + 1) * 128], ident_bf)
                            nc.scalar.copy(at_half[:, nn * 32 + i, :], ptr)

                    # output projection for this half: out[b, n*1024+j] = a @ Wp
                    for jc in range(2):
                        pss = []
                        for _pi in range(2):
                            ps_n = psum_mm.tile([128, 512], F32, tag="mm")
                            pss.append(ps_n)
                        for sub in range(4):
                            wpt = wst_p.tile([128, 8, 512], BF16, tag="w8")
                            for i8 in range(8):
                                i = sub * 8 + i8
                                nc.sync.dma_start(
                                    out=wpt[:, i8, :],
                                    in_=wp_d[i * 128:(i + 1) * 128, jc * 512:(jc + 1) * 512],
                                )
                            for nn in range(2):
                                for i8 in range(8):
                                    i = sub * 8 + i8
                                    nc.tensor.matmul(
                                        pss[nn],
                                        at_half[:, nn * 32 + i, :],
                                        wpt[:, i8, :],
                                        start=(sub == 0 and i8 == 0),
                                        stop=(sub == 3 and i8 == 7),
                                    )
                        for nn in range(2):
                            n = half * 2 + nn
                            osb = outsb_p.tile([128, 512], F32, tag="osb")
                            nc.scalar.copy(osb, pss[nn])
                            nc.sync.dma_start(
                                out=out_d[b0:b0 + 128, n * DIM + jc * 512:n * DIM + (jc + 1) * 512],
                                in_=osb,
                            )
    import bass_rust as _bass_rust
    _bass_rust.move_matmul_waits_to_ldweights(nc.m)
    _bass_rust.generate_event_semaphores(nc)
    return nc


def build_mix_consts(Wl, Ww, bl, bw):
    """Host-built block-diagonal mix matrices for the transposed
    [(head, m), b] attention space. Row/col order is head-major: r = g*8+m."""
    m1 = np.zeros((128, 128), np.float32)   # [(h,m), (g,m)] = Wl[h,g]
    m2 = np.zeros((128, 128), np.float32)   # [(g,m), (g2,m)] = Ww[g,g2]
    for m in range(NTOK):
        for h in range(H):
            for g in range(H):
                m1[h * 8 + m, g * 8 + m] = Wl[h, g]
                m2[h * 8 + m, g * 8 + m] = Ww[h, g]
    onesd = np.zeros((128, 16), np.float32)  # [(g,m), g'] = (g == g')
    for g in range(H):
        for m in range(NTOK):
            onesd[g * 8 + m, g] = 1.0
    wm = np.concatenate([m1, m2, onesd], axis=1).astype(ml_dtypes.bfloat16)
    wb = np.zeros((128, 2), np.float32)
    for g in range(H):
        for m in range(NTOK):
            wb[g * 8 + m, 0] = bl[g]
            wb[g * 8 + m, 1] = bw[g]
    return wm, wb


def _to_bf16(a):
    return np.asarray(a, dtype=np.float32).astype(ml_dtypes.bfloat16)


def kernel(**inputs) -> np.ndarray:
    global LAST_RESULT, LAST_TIMES
    x = np.ascontiguousarray(np.asarray(inputs["x"], dtype=np.float32))
    Wl = np.asarray(inputs["Wl"], np.float32)
    Ww = np.asarray(inputs["Ww"], np.float32)
    bl = np.asarray(inputs["bl"], np.float32)
    bw = np.asarray(inputs["bw"], np.float32)

    gamma = np.asarray(inputs["gamma"], np.float32)
    beta = np.asarray(inputs["beta"], np.float32)
    for name in ("bq", "bk", "bv", "bp"):
        assert not np.any(np.asarray(inputs[name], np.float32)), f"{name} != 0 unsupported"
    assert np.all(gamma == 1.0) and not np.any(beta), "non-identity LN unsupported"

    nc = build_program()
    wm, wb = build_mix_consts(Wl, Ww, bl, bw)
    xb16 = x.astype(ml_dtypes.bfloat16)

    in_maps = [
        {
            "x": xb16[c * BPC:(c + 1) * BPC],
            "wq": _to_bf16(inputs["Wq"]),
            "wk": _to_bf16(inputs["Wk"]),
            "wv": _to_bf16(inputs["Wv"]),
            "wp": _to_bf16(inputs["Wp"]),
            "wm": wm,
            "wb": wb,
        }
        for c in range(N_CORES)
    ]
    res = run_bass_kernel_spmd(nc, in_maps, list(range(N_CORES)))
    LAST_RESULT = res
    if os.environ.get("BASS_BENCH"):
        import time as _time
        LAST_TIMES = []
        for _ in range(int(os.environ.get("BASS_BENCH_REPEATS", "3"))):
            t0 = _time.time()
            run_bass_kernel_spmd(nc, in_maps, list(range(N_CORES)))
            LAST_TIMES.append(_time.time() - t0)
    out = np.concatenate(
        [np.asarray(res.results[c]["out"]) for c in range(N_CORES)], axis=0
    ).astype(np.float32)
    return out
